# revision 22
# baseline (speedup 1.0000x reference)
"""GroupedMLP (MoE) kernel for 8 TRN2 NeuronCores.

Expert-parallel: expert i -> core i. Each core computes, for its expert's
2048-token block X [T=2048, H=2048]:
    fc1 = X @ w1.T          # w1 [8192, 2048]
    inter = silu(a) * b     # a,b = split(fc1, 2, axis=-1)
    out = inter @ w2.T      # w2 [2048, 4096]

Device-side everything is computed transposed (contraction dim on SBUF
partitions):
  phase 1: fc1T[m,t] = sum_k w1T_tile[k,m].T @ xT[k,t]   (PSUM, fp32)
           interT = silu(fc1T[a-rows]) * fc1T[b-rows]    (bf16, SBUF)
  phase 2: yT[h,t]  = sum_f w2T_tile[f,h].T @ interT[f,t]
Matmuls run in bf16 (full-rate on the PE), accumulation in fp32 PSUM.
Two passes of 1024 tokens each so interT + x + weight tiles fit in SBUF.

v2 changes vs baseline (1362 us -> target ~1335):
 - startup: no warmup burn; DMAs priority-ordered on the sync ring so the
   first real matmul starts right at engine-preamble end (~6.5us instead
   of ~13.5us) and m=0..1 are fed just-in-time.
 - phase-2 w2 / output stores / pass-1 x prefetch all live on the scalar
   ring so the sync ring carries only the latency-critical w1 stream.
 - post-compile surgery drops the second InstLdweights of each
   (lhs, tb0/tb1) pair (PE reuses loaded weights; validated on HW). 25%
   fewer PE instructions -> proportionally fewer periodic iq-fetch stalls.
 - final output copy/store split finer to shorten the drain tail.

Host side shards/transposes/casts inputs and transposes the output back.
"""

import numpy as np
import ml_dtypes
from contextlib import ExitStack

P = 128
H = 2048          # hidden size
F = 4096          # ffn hidden (one GLU half)
T = 2048          # tokens per expert
NE = 8            # experts == cores
TPASS = 1024      # tokens per pass
NPASS = T // TPASS
NT = 512          # matmul moving free dim (one PSUM bank of fp32)

_BF16 = ml_dtypes.bfloat16

_nc_cache = {}


def _ldw_sig(inst):
    ap = inst.ins[0]
    return (
        ap.memref,
        ap.offset,
        str(ap.ap),
        str(ap.dtype),
        str(inst.tile_size),
        str(inst.tile_position),
        str(inst.perf_mode),
        str(inst.is_transpose),
    )


def _dedup_ldweights(nc):
    """Remove InstLdweights that reload the exact weights the PE already
    holds (only when separated from the previous identical load by nothing
    but matmuls, and carrying no semaphore waits/updates)."""
    removed = 0
    for b in nc.m.functions[0].blocks:
        last_sig = None
        to_remove = []
        for inst in b.instructions:
            if str(getattr(inst, "engine", "")) != "EngineType.PE":
                continue
            tn = type(inst).__name__
            if tn == "InstLdweights":
                sig = _ldw_sig(inst)
                if sig == last_sig and not inst.has_wait() and not inst.has_update():
                    to_remove.append(inst)
                else:
                    last_sig = sig
            elif tn == "InstMatmult":
                pass
            else:
                last_sig = None
        il = b.instructions
        for inst in to_remove:
            il.remove(inst)
            removed += 1
    return removed


def _build_nc():
    import concourse.mybir as mybir
    import concourse.tile as tile
    from concourse import bacc

    nc = bacc.Bacc("TRN2", target_bir_lowering=False, debug=False)
    bf16 = mybir.dt.bfloat16
    f32 = mybir.dt.float32
    Silu = mybir.ActivationFunctionType.Silu

    # Per-core shards, host-prearranged so every DMA is contiguous:
    #  xr[kk, p, t]        = X.T[kk*128+p, t]                       (bf16)
    #  w1r[m, p, kk, c]    = w1.T[kk*128+p, mcol(m,c)]              (bf16)
    #       mcol(m,c) = m*128+c for c<128 (silu half), 4096+m*128+(c-128) else
    #  w2r[h2, p, f, c]    = w2.T[f*128+p, h2*256+c]                (bf16)
    #  yr[hh, p, t]        = out.T[hh*128+p, t]                     (fp32)
    xr = nc.declare_dram_parameter("xr", [16, P, T], bf16, isOutput=False)
    w1r = nc.declare_dram_parameter("w1r", [32, P, 16, 256], bf16, isOutput=False)
    w2r = nc.declare_dram_parameter("w2r", [8, P, 32, 256], bf16, isOutput=False)
    yr = nc.declare_dram_parameter("yr", [16, P, T], f32, isOutput=True)

    with tile.TileContext(nc) as tc, ExitStack() as ctx:
        xpool = ctx.enter_context(tc.tile_pool(name="x", bufs=1))
        ipool = ctx.enter_context(tc.tile_pool(name="inter", bufs=1))
        w1pool = ctx.enter_context(tc.tile_pool(name="w1", bufs=3))
        w2pool = ctx.enter_context(tc.tile_pool(name="w2", bufs=2))
        tpool = ctx.enter_context(tc.tile_pool(name="tmp", bufs=2))
        opool = ctx.enter_context(tc.tile_pool(name="osb", bufs=3))
        psum = ctx.enter_context(tc.tile_pool(name="psum", bufs=2, space="PSUM"))

        # X.T resident in SBUF (8 MB bf16) as per-pass column blocks.
        xsb = [[None] * 16 for _ in range(NPASS)]

        def load_x_tile(ps, kk, eng):
            off = ps * TPASS
            xt = xpool.tile([P, TPASS], bf16, tag=f"x{ps}_{kk}", bufs=1,
                            name=f"x{ps}_{kk}")
            eng.dma_start(xt[:], xr[kk][:, off : off + TPASS])
            xsb[ps][kk] = xt

        def w1_dma(t, m):
            # two half-loads so kk=0..7 matmuls can start on the first half
            nc.sync.dma_start(t[:, 0:8, :], w1r[m][:, 0:8, :])
            nc.sync.dma_start(t[:, 8:16, :], w1r[m][:, 8:16, :])

        # Startup. Doorbell issue costs ~650ns each, so latency-critical
        # transfers are spread over three engine rings, each in deadline
        # order: sync carries w1 (first matmul gate), vector carries x kk=0-7
        # (after the warmup memset), scalar carries x kk=8-15.
        # m=0 and m=1 run with interleaved kk-loops (below), so their w1
        # tiles stream in matching interleaved chunks.
        w1m_first = w1pool.tile([P, 16, 256], bf16, tag="w1m")
        w1m_second = w1pool.tile([P, 16, 256], bf16, tag="w1m")
        for lo, hi in ((0, 2), (2, 4), (4, 8), (8, 16)):
            nc.sync.dma_start(w1m_first[:, lo:hi, :], w1r[0][:, lo:hi, :])
            nc.sync.dma_start(w1m_second[:, lo:hi, :], w1r[1][:, lo:hi, :])
        w1q = [w1m_first, w1m_second]   # FIFO of prefetched w1 tiles

        # Warmup: the engine preambles end ~6.5us but the first operands only
        # land ~11us; a few scratch matmuls fill that window so the HAM
        # clock gate is warm when real work starts.
        warm = xpool.tile([P, NT], bf16, tag="warm", bufs=1, name="warm")
        nc.vector.memset(warm[:], 0.0)
        pw = psum.tile([P, TPASS], f32, tag="pa", name="pwarm")
        for i in range(6):
            nc.tensor.matmul(
                pw[:, 0:NT], warm[:, 0:128], warm[:], start=True, stop=True
            )

        for kk in range(16):
            load_x_tile(0, kk, nc.scalar)

        # interT tiles: 32 x [128, TPASS] bf16 (8 MB), reused across passes.
        inter = [
            ipool.tile([P, TPASS], bf16, tag=f"i{m}", bufs=1, name=f"inter{m}")
            for m in range(32)
        ]

        def mm_quad(ps, w1m, pa, pb, kk):
            la = w1m[:, kk, 0:128]
            lb = w1m[:, kk, 128:256]
            st = kk == 0
            sp = kk == 15
            # consecutive matmuls share the stationary operand; the
            # post-compile pass (optional) drops the duplicate LDWEIGHTS
            for lhs, pd in ((la, pa), (lb, pb)):
                for tb in range(TPASS // NT):
                    r = xsb[ps][kk][:, tb * NT : (tb + 1) * NT]
                    nc.tensor.matmul(
                        pd[:, tb * NT : (tb + 1) * NT], lhs, r,
                        start=st, stop=sp,
                    )

        def glu(m, pa, pb):
            tmp = tpool.tile([P, TPASS], f32, tag="tmp")
            nc.scalar.activation(tmp[:], pa[:], Silu)
            nc.vector.tensor_mul(inter[m][:], tmp[:], pb[:])

        def prefetch_w1(gidx):
            if gidx < NPASS * 32:
                w1n = w1pool.tile([P, 16, 256], bf16, tag="w1m", name=f"w1p{gidx}")
                w1_dma(w1n, gidx % 32)
                w1q.append(w1n)

        for ps in range(NPASS):
            off = ps * TPASS

            # ---- phase 1: fc1T + GLU -> interT ----
            w2pre = {}

            # m=0 and m=1 run with interleaved kk-loops: each x tile feeds 8
            # matmuls instead of 4, halving the startup x-arrival rate the PE
            # needs and hiding DMA jitter. m=0 finishes its last 3 kk solo so
            # its silu/mul (which free the PSUM slots m=2 reuses) complete
            # behind m=1's tail matmuls -- no transition bubble. Uses all 8
            # PSUM banks (2x pa + 2x pb).
            ILV = 13
            w1m_a = w1q.pop(0)
            w1m_b = w1q.pop(0)
            pa_a = psum.tile([P, TPASS], f32, tag="pa", name=f"paa{ps}")
            pb_a = psum.tile([P, TPASS], f32, tag="pb", name=f"pba{ps}")
            pa_b = psum.tile([P, TPASS], f32, tag="pa", name=f"pab{ps}")
            pb_b = psum.tile([P, TPASS], f32, tag="pb", name=f"pbb{ps}")
            for kk in range(ILV):
                if kk == 2:
                    prefetch_w1(ps * 32 + 2)
                if kk == 6:
                    prefetch_w1(ps * 32 + 3)
                mm_quad(ps, w1m_a, pa_a, pb_a, kk)
                mm_quad(ps, w1m_b, pa_b, pb_b, kk)
            for kk in range(ILV, 16):
                mm_quad(ps, w1m_a, pa_a, pb_a, kk)
            glu(0, pa_a, pb_a)
            for kk in range(ILV, 16):
                mm_quad(ps, w1m_b, pa_b, pb_b, kk)
            glu(1, pa_b, pb_b)

            for m in range(2, 32):
                # prefetch w1m two tiles ahead (crossing into the next pass:
                # w1 data is identical for both passes)
                prefetch_w1(ps * 32 + m + 2)
                if ps + 1 < NPASS and m == 16:
                    for kk in range(16):
                        load_x_tile(ps + 1, kk, nc.scalar)
                if m == 3:
                    # prefetch the first two w2 tiles on the scalar ring
                    for h2 in range(2):
                        w2m = w2pool.tile(
                            [P, 32, 256], bf16, tag="w2m", name=f"w2m_{ps}_{h2}"
                        )
                        nc.scalar.dma_start(w2m[:], w2r[h2])
                        w2pre[h2] = w2m
                w1m = w1q.pop(0)
                pa = psum.tile([P, TPASS], f32, tag="pa")
                pb = psum.tile([P, TPASS], f32, tag="pb")
                for kk in range(16):
                    mm_quad(ps, w1m, pa, pb, kk)
                glu(m, pa, pb)

            # ---- phase 2: yT = w2T.T @ interT ----
            for h2 in range(8):
                if h2 in w2pre:
                    w2m = w2pre[h2]
                else:
                    w2m = w2pool.tile([P, 32, 256], bf16, tag="w2m")
                    nc.scalar.dma_start(w2m[:], w2r[h2])
                for hh in range(2):
                    last = ps == NPASS - 1 and h2 == 7 and hh == 1
                    po = psum.tile([P, TPASS], f32, tag="pa")  # reuse pa slots
                    osb = opool.tile([P, TPASS], f32, tag="osb")
                    # copy on DVE (idle in phase 2) so ScalarE never swaps
                    # activation tables; split halves to overlap copy and store.
                    # For the very last block the two 512-col halves (separate
                    # PSUM banks) run as sequential accumulation groups, so
                    # half 0's copy+store overlaps half 1's matmuls and the
                    # kernel tail shrinks to one half's drain.
                    def emit_group(tbs):
                        for f in range(32):
                            lw = w2m[:, f, hh * 128 : (hh + 1) * 128]
                            for tb in tbs:
                                nc.tensor.matmul(
                                    po[:, tb * NT : (tb + 1) * NT],
                                    lw,
                                    inter[f][:, tb * NT : (tb + 1) * NT],
                                    start=f == 0,
                                    stop=f == 31,
                                )

                    def drain(tb):
                        sl = slice(tb * NT, (tb + 1) * NT)
                        nc.vector.tensor_copy(osb[:, sl], po[:, sl])
                        nc.scalar.dma_start(
                            yr[h2 * 2 + hh][:, off + tb * NT : off + (tb + 1) * NT],
                            osb[:, sl],
                        )

                    if not last:
                        emit_group([0, 1])
                        drain(0)
                        drain(1)
                    else:
                        emit_group([0])
                        drain(0)
                        emit_group([1])
                        drain(1)
    nc.compile()
    import os
    if os.environ.get("KERNEL_DEDUP"):
        _dedup_ldweights(nc)
    return nc


def _prep_core_inputs(x, w1_i, w2_i):
    """Host-side reshape/cast of one expert's shard into DMA-friendly layouts."""
    xT = np.ascontiguousarray(x.T)                       # [H, T]
    xr = xT.reshape(16, P, T).astype(_BF16)

    w1T = w1_i.T                                         # [H, 8192]
    a = w1T[:, :F].reshape(H, 32, P)
    b = w1T[:, F:].reshape(H, 32, P)
    cat = np.concatenate([a, b], axis=2)                 # [H, 32, 256]
    w1r = np.ascontiguousarray(
        cat.reshape(16, P, 32, 256).transpose(2, 1, 0, 3)
    ).astype(_BF16)                                      # [32, P, 16, 256]

    w2T = w2_i.T                                         # [F, H]
    w2r = np.ascontiguousarray(
        w2T.reshape(32, P, 8, 256).transpose(2, 1, 0, 3)
    ).astype(_BF16)                                      # [8, P, 32, 256]
    return {"xr": xr, "w1r": w1r, "w2r": w2r}


_last_results = None


def kernel(permuted_hidden_states, tokens_per_expert, w1, w2):
    global _last_results
    x = np.asarray(permuted_hidden_states, dtype=np.float32)
    counts = np.asarray(tokens_per_expert).astype(np.int64)
    w1 = np.asarray(w1, dtype=np.float32)
    w2 = np.asarray(w2, dtype=np.float32)

    if not (counts.shape == (NE,) and np.all(counts == T)):
        return _numpy_fallback(x, counts, w1, w2)

    from concourse.bass_utils import run_bass_kernel_spmd

    if "nc" not in _nc_cache:
        _nc_cache["nc"] = _build_nc()
    nc = _nc_cache["nc"]

    in_maps = [
        _prep_core_inputs(x[i * T : (i + 1) * T], w1[i], w2[i]) for i in range(NE)
    ]
    import os

    res = run_bass_kernel_spmd(
        nc,
        in_maps,
        core_ids=list(range(NE)),
        trace=bool(os.environ.get("BASS_TRACE")),
    )
    _last_results = res

    out = np.empty((NE * T, H), dtype=np.float32)
    for i in range(NE):
        yT = res.results[i]["yr"].reshape(H, T)
        out[i * T : (i + 1) * T] = yT.T
    return out


def _numpy_fallback(x, counts, w1, w2):
    outs = []
    start = 0
    for i in range(counts.shape[0]):
        n = int(counts[i])
        if n == 0:
            continue
        xi = x[start : start + n]
        fc1 = xi @ w1[i].T
        a, b = fc1[:, :F], fc1[:, F:]
        inter = (a / (1.0 + np.exp(-a))) * b
        outs.append(inter @ w2[i].T)
        start += n
    return np.concatenate(outs, axis=0).astype(np.float32)


# revision 23
# speedup vs baseline: 1.1922x; 1.1922x over previous
"""GroupedMLP (MoE) kernel for 8 TRN2 NeuronCores.

Expert-parallel: expert i -> core i. Each core computes, for its expert's
2048-token block X [T=2048, H=2048]:
    fc1 = X @ w1.T          # w1 [8192, 2048]
    inter = silu(a) * b     # a,b = split(fc1, 2, axis=-1)
    out = inter @ w2.T      # w2 [2048, 4096]

Device-side everything is computed transposed (contraction dim on SBUF
partitions):
  phase 1: fc1T[m,t] = sum_k w1T_tile[k,m].T @ xT[k,t]   (PSUM, fp32)
           interT = silu(fc1T[a-rows]) * fc1T[b-rows]    (bf16, SBUF)
  phase 2: yT[h,t]  = sum_f w2T_tile[f,h].T @ interT[f,t]
Matmuls run in bf16 (full-rate on the PE), accumulation in fp32 PSUM.
Two passes of 1024 tokens each so interT + x + weight tiles fit in SBUF.

v2 changes vs baseline (1362 us -> target ~1335):
 - startup: no warmup burn; DMAs priority-ordered on the sync ring so the
   first real matmul starts right at engine-preamble end (~6.5us instead
   of ~13.5us) and m=0..1 are fed just-in-time.
 - phase-2 w2 / output stores / pass-1 x prefetch all live on the scalar
   ring so the sync ring carries only the latency-critical w1 stream.
 - post-compile surgery drops the second InstLdweights of each
   (lhs, tb0/tb1) pair (PE reuses loaded weights; validated on HW). 25%
   fewer PE instructions -> proportionally fewer periodic iq-fetch stalls.
 - final output copy/store split finer to shorten the drain tail.

Host side shards/transposes/casts inputs and transposes the output back.
"""

import numpy as np
import ml_dtypes
from contextlib import ExitStack

P = 128
H = 2048          # hidden size
F = 4096          # ffn hidden (one GLU half)
T = 2048          # tokens per expert
NE = 8            # experts == cores
TPASS = 1024      # tokens per pass
NPASS = T // TPASS
NT = 512          # matmul moving free dim (one PSUM bank of fp32)

_BF16 = ml_dtypes.bfloat16

_nc_cache = {}


def _ldw_sig(inst):
    ap = inst.ins[0]
    return (
        ap.memref,
        ap.offset,
        str(ap.ap),
        str(ap.dtype),
        str(inst.tile_size),
        str(inst.tile_position),
        str(inst.perf_mode),
        str(inst.is_transpose),
    )


def _dedup_ldweights(nc):
    """Remove InstLdweights that reload the exact weights the PE already
    holds (only when separated from the previous identical load by nothing
    but matmuls, and carrying no semaphore waits/updates)."""
    removed = 0
    for b in nc.m.functions[0].blocks:
        last_sig = None
        to_remove = []
        for inst in b.instructions:
            if str(getattr(inst, "engine", "")) != "EngineType.PE":
                continue
            tn = type(inst).__name__
            if tn == "InstLdweights":
                sig = _ldw_sig(inst)
                if sig == last_sig and not inst.has_wait() and not inst.has_update():
                    to_remove.append(inst)
                else:
                    last_sig = sig
            elif tn == "InstMatmult":
                pass
            else:
                last_sig = None
        il = b.instructions
        for inst in to_remove:
            il.remove(inst)
            removed += 1
    return removed


def _build_nc():
    import concourse.mybir as mybir
    import concourse.tile as tile
    from concourse import bacc

    nc = bacc.Bacc("TRN2", target_bir_lowering=False, debug=False)
    bf16 = mybir.dt.bfloat16
    f32 = mybir.dt.float32
    Silu = mybir.ActivationFunctionType.Silu

    # Per-core shards, host-prearranged so every DMA is contiguous:
    #  xr[kk, p, t]        = X.T[kk*128+p, t]                       (bf16)
    #  w1r[m, p, kk, c]    = w1.T[kk*128+p, mcol(m,c)]              (bf16)
    #       mcol(m,c) = m*128+c for c<128 (silu half), 4096+m*128+(c-128) else
    #  w2r[h2, p, f, c]    = w2.T[f*128+p, h2*256+c]                (bf16)
    #  yr[hh, p, t]        = out.T[hh*128+p, t]                     (fp32)
    xr = nc.declare_dram_parameter("xr", [16, P, T], bf16, isOutput=False)
    w1r = nc.declare_dram_parameter("w1r", [32, P, 16, 256], bf16, isOutput=False)
    w2r = nc.declare_dram_parameter("w2r", [8, P, 32, 256], bf16, isOutput=False)
    yr = nc.declare_dram_parameter("yr", [16, P, T], f32, isOutput=True)

    with tile.TileContext(nc) as tc, ExitStack() as ctx:
        xpool = ctx.enter_context(tc.tile_pool(name="x", bufs=1))
        ipool = ctx.enter_context(tc.tile_pool(name="inter", bufs=1))
        w1pool = ctx.enter_context(tc.tile_pool(name="w1", bufs=3))
        w2pool = ctx.enter_context(tc.tile_pool(name="w2", bufs=2))
        tpool = ctx.enter_context(tc.tile_pool(name="tmp", bufs=2))
        opool = ctx.enter_context(tc.tile_pool(name="osb", bufs=3))
        psum = ctx.enter_context(tc.tile_pool(name="psum", bufs=2, space="PSUM"))

        # X.T resident in SBUF (8 MB bf16) as per-pass column blocks.
        xsb = [[None] * 16 for _ in range(NPASS)]

        def load_x_tile(ps, kk, eng):
            off = ps * TPASS
            xt = xpool.tile([P, TPASS], bf16, tag=f"x{ps}_{kk}", bufs=1,
                            name=f"x{ps}_{kk}")
            eng.dma_start(xt[:], xr[kk][:, off : off + TPASS])
            xsb[ps][kk] = xt

        def w1_dma(t, m):
            # two half-loads so kk=0..7 matmuls can start on the first half
            nc.sync.dma_start(t[:, 0:8, :], w1r[m][:, 0:8, :])
            nc.sync.dma_start(t[:, 8:16, :], w1r[m][:, 8:16, :])

        # Startup. Doorbell issue costs ~650ns each, so latency-critical
        # transfers are spread over three engine rings, each in deadline
        # order: sync carries w1 (first matmul gate), vector carries x kk=0-7
        # (after the warmup memset), scalar carries x kk=8-15.
        w1m_first = w1pool.tile([P, 16, 256], bf16, tag="w1m")
        nc.sync.dma_start(w1m_first[:, 0:2, :], w1r[0][:, 0:2, :])
        nc.sync.dma_start(w1m_first[:, 2:4, :], w1r[0][:, 2:4, :])
        nc.sync.dma_start(w1m_first[:, 4:8, :], w1r[0][:, 4:8, :])
        nc.sync.dma_start(w1m_first[:, 8:16, :], w1r[0][:, 8:16, :])
        w1m_second = w1pool.tile([P, 16, 256], bf16, tag="w1m")
        w1_dma(w1m_second, 1)
        w1q = [w1m_first, w1m_second]   # FIFO of prefetched w1 tiles

        # Warmup: the engine preambles end ~6.5us but the first operands only
        # land ~11us; a few scratch matmuls fill that window so the HAM
        # clock gate is warm when real work starts.
        warm = xpool.tile([P, NT], bf16, tag="warm", bufs=1, name="warm")
        nc.vector.memset(warm[:], 0.0)
        pw = psum.tile([P, TPASS], f32, tag="pa", name="pwarm")
        for i in range(6):
            nc.tensor.matmul(
                pw[:, 0:NT], warm[:, 0:128], warm[:], start=True, stop=True
            )

        for kk in range(16):
            load_x_tile(0, kk, nc.scalar)

        # interT tiles: 32 x [128, TPASS] bf16 (8 MB), reused across passes.
        inter = [
            ipool.tile([P, TPASS], bf16, tag=f"i{m}", bufs=1, name=f"inter{m}")
            for m in range(32)
        ]

        for ps in range(NPASS):
            off = ps * TPASS

            # ---- phase 1: fc1T + GLU -> interT ----
            w2pre = {}
            for m in range(32):
                # prefetch w1m two tiles ahead (crossing into the next pass:
                # w1 data is identical for both passes)
                gidx = ps * 32 + m + 2
                if gidx < NPASS * 32:
                    w1n = w1pool.tile([P, 16, 256], bf16, tag="w1m")
                    w1_dma(w1n, gidx % 32)
                    w1q.append(w1n)
                if ps + 1 < NPASS and m == 16:
                    for kk in range(16):
                        load_x_tile(ps + 1, kk, nc.scalar)
                if m == 3:
                    # prefetch the first two w2 tiles on the scalar ring
                    for h2 in range(2):
                        w2m = w2pool.tile(
                            [P, 32, 256], bf16, tag="w2m", name=f"w2m_{ps}_{h2}"
                        )
                        nc.scalar.dma_start(w2m[:], w2r[h2])
                        w2pre[h2] = w2m
                w1m = w1q.pop(0)
                pa = psum.tile([P, TPASS], f32, tag="pa")
                pb = psum.tile([P, TPASS], f32, tag="pb")
                for kk in range(16):
                    la = w1m[:, kk, 0:128]
                    lb = w1m[:, kk, 128:256]
                    st = kk == 0
                    sp = kk == 15
                    # consecutive matmuls share the stationary operand; the
                    # post-compile pass drops the duplicate LDWEIGHTS
                    for lhs, pd in ((la, pa), (lb, pb)):
                        for tb in range(TPASS // NT):
                            r = xsb[ps][kk][:, tb * NT : (tb + 1) * NT]
                            nc.tensor.matmul(
                                pd[:, tb * NT : (tb + 1) * NT], lhs, r,
                                start=st, stop=sp,
                            )
                tmp = tpool.tile([P, TPASS], f32, tag="tmp")
                nc.scalar.activation(tmp[:], pa[:], Silu)
                nc.vector.tensor_mul(inter[m][:], tmp[:], pb[:])

            # ---- phase 2: yT = w2T.T @ interT ----
            for h2 in range(8):
                if h2 in w2pre:
                    w2m = w2pre[h2]
                else:
                    w2m = w2pool.tile([P, 32, 256], bf16, tag="w2m")
                    nc.scalar.dma_start(w2m[:], w2r[h2])
                for hh in range(2):
                    last = ps == NPASS - 1 and h2 == 7 and hh == 1
                    po = psum.tile([P, TPASS], f32, tag="pa")  # reuse pa slots
                    osb = opool.tile([P, TPASS], f32, tag="osb")
                    # copy on DVE (idle in phase 2) so ScalarE never swaps
                    # activation tables; split halves to overlap copy and store.
                    # For the very last block the two 512-col halves (separate
                    # PSUM banks) run as sequential accumulation groups, so
                    # half 0's copy+store overlaps half 1's matmuls and the
                    # kernel tail shrinks to one half's drain.
                    def emit_group(tbs):
                        for f in range(32):
                            lw = w2m[:, f, hh * 128 : (hh + 1) * 128]
                            for tb in tbs:
                                nc.tensor.matmul(
                                    po[:, tb * NT : (tb + 1) * NT],
                                    lw,
                                    inter[f][:, tb * NT : (tb + 1) * NT],
                                    start=f == 0,
                                    stop=f == 31,
                                )

                    def drain(tb):
                        sl = slice(tb * NT, (tb + 1) * NT)
                        nc.vector.tensor_copy(osb[:, sl], po[:, sl])
                        nc.scalar.dma_start(
                            yr[h2 * 2 + hh][:, off + tb * NT : off + (tb + 1) * NT],
                            osb[:, sl],
                        )

                    if not last:
                        emit_group([0, 1])
                        drain(0)
                        drain(1)
                    else:
                        emit_group([0])
                        drain(0)
                        emit_group([1])
                        drain(1)
    nc.compile()
    import os
    if os.environ.get("KERNEL_DEDUP"):
        _dedup_ldweights(nc)
    return nc


def _prep_core_inputs(x, w1_i, w2_i):
    """Host-side reshape/cast of one expert's shard into DMA-friendly layouts."""
    xT = np.ascontiguousarray(x.T)                       # [H, T]
    xr = xT.reshape(16, P, T).astype(_BF16)

    w1T = w1_i.T                                         # [H, 8192]
    a = w1T[:, :F].reshape(H, 32, P)
    b = w1T[:, F:].reshape(H, 32, P)
    cat = np.concatenate([a, b], axis=2)                 # [H, 32, 256]
    w1r = np.ascontiguousarray(
        cat.reshape(16, P, 32, 256).transpose(2, 1, 0, 3)
    ).astype(_BF16)                                      # [32, P, 16, 256]

    w2T = w2_i.T                                         # [F, H]
    w2r = np.ascontiguousarray(
        w2T.reshape(32, P, 8, 256).transpose(2, 1, 0, 3)
    ).astype(_BF16)                                      # [8, P, 32, 256]
    return {"xr": xr, "w1r": w1r, "w2r": w2r}


_last_results = None


def kernel(permuted_hidden_states, tokens_per_expert, w1, w2):
    global _last_results
    x = np.asarray(permuted_hidden_states, dtype=np.float32)
    counts = np.asarray(tokens_per_expert).astype(np.int64)
    w1 = np.asarray(w1, dtype=np.float32)
    w2 = np.asarray(w2, dtype=np.float32)

    if not (counts.shape == (NE,) and np.all(counts == T)):
        return _numpy_fallback(x, counts, w1, w2)

    from concourse.bass_utils import run_bass_kernel_spmd

    if "nc" not in _nc_cache:
        _nc_cache["nc"] = _build_nc()
    nc = _nc_cache["nc"]

    in_maps = [
        _prep_core_inputs(x[i * T : (i + 1) * T], w1[i], w2[i]) for i in range(NE)
    ]
    import os

    res = run_bass_kernel_spmd(
        nc,
        in_maps,
        core_ids=list(range(NE)),
        trace=bool(os.environ.get("BASS_TRACE")),
    )
    _last_results = res

    out = np.empty((NE * T, H), dtype=np.float32)
    for i in range(NE):
        yT = res.results[i]["yr"].reshape(H, T)
        out[i * T : (i + 1) * T] = yT.T
    return out


def _numpy_fallback(x, counts, w1, w2):
    outs = []
    start = 0
    for i in range(counts.shape[0]):
        n = int(counts[i])
        if n == 0:
            continue
        xi = x[start : start + n]
        fc1 = xi @ w1[i].T
        a, b = fc1[:, :F], fc1[:, F:]
        inter = (a / (1.0 + np.exp(-a))) * b
        outs.append(inter @ w2[i].T)
        start += n
    return np.concatenate(outs, axis=0).astype(np.float32)
